# revision 10
# baseline (speedup 1.0000x reference)
"""Bernstein-basis GNN kernel for Trainium2 (8 NeuronCores), v3.

out[k] = sum_m coeffs[k,m] * Ltilde^m @ x,  Ltilde = 0.5*I - 0.5*A_norm.

Strategy vs the v1 kernel: the per-chunk [128,1] indirect gathers (994ns
SWDGE fixed cost each, ~17k instructions -> 20ms of Pool time) are replaced
by bulk dma_gather instructions. dma_gather indices are int16 (<32768), so
the gather table is laid out as 8 slabs of 16384 padded rows (one per core)
giving 4 "quarters" of 32768 rows, each owned by a core PAIR. Destination
rows are packed into windows of <=32 rows such that each window has <=128
edges per quarter; a window's quarter-q edges form one 128-slot chunk
gathered positionally (slot i%128, column i//128) by a compact per-
(batch,quarter) dma_gather of 16 windows at a time. Table rows are padded
to 64 f32 (256B, the dma_gather element quantum). The segment-sum is a
matmul per chunk: psum[48, 32w:32w+32] += V_chunk[:, :48].T @ S_chunk with
S fp16 [128,32] (one-hot x edge_val; fp16 S costs ~1e-4 rel err). Psum
batches of 512 rows drain exactly as v1: PE-transpose to row-major, affine
DMA into the AllGather slab and into the Bernstein-combination layout.
"""

import os
import sys
import numpy as np
from math import comb

for _p in ("/opt/trn_rl_repo", "/root/.axon_site/_ro/trn_rl_repo"):
    if os.path.isdir(_p) and _p not in sys.path:
        sys.path.insert(0, _p)

import concourse.bass as bass
import concourse.bacc as bacc
import concourse.tile as tile
import concourse.mybir as mybir
import concourse.bass_utils as bass_utils
from concourse.masks import make_identity

NCORES = 8
K = 10            # polynomial order -> K+1 basis outputs
C = 48
CP = 64           # padded table row width (f32) = 256B gather element
W2 = 32           # max dest rows per window (= S free width)
SLOTS = 128       # slots per chunk
NQ = 4            # index quarters (core pairs)
NV = 16384        # vrows per core slab (2 slabs = 1 quarter = 32768)
NVG = NCORES * NV
BW = 16           # windows per psum batch (512 rows = 1 psum bank)
PSF = BW * W2     # 512
RS8 = 8
GC = 128
CBR = RS8 * GC    # 1024 combination batch rows
KD = K + 1
P88 = RS8 * KD    # 88

LAST_EXEC_TIME_NS = None
LAST_RESULTS = None


def _bern_coeff_matrix(k):
    m = np.zeros((k + 1, k + 1), dtype=np.float64)
    for i in range(k + 1):
        for j in range(k - i + 1):
            m[i, i + j] = ((-1) ** j) * comb(k, i) * comb(k - i, j)
    return m


def _pack_windows(pc):
    """Pack rows into windows of <=W2 rows with per-quarter edge counts
    <=SLOTS. pc: [R, 4] per-row per-quarter counts. Best-fit-decreasing over
    a pool of open bins (vectorized fit test)."""
    R = pc.shape[0]
    order = np.argsort(-pc.max(1), kind="stable")
    MAXOPEN = 64
    acc = np.zeros((MAXOPEN, 4), np.int64)      # per-bin quarter loads
    cnt = np.zeros(MAXOPEN, np.int64)           # rows per bin
    bins = [[] for _ in range(MAXOPEN)]
    closed = []
    for r in order:
        fits = (cnt < W2) & np.all(acc + pc[r] <= SLOTS, axis=1)
        if fits.any():
            load = acc.sum(1)
            load[~fits] = -1
            i = int(np.argmax(load))
        else:
            i = int(np.argmax(acc.sum(1)))      # close fullest, reuse slot
            closed.append(bins[i])
            bins[i] = []
            acc[i] = 0
            cnt[i] = 0
        bins[i].append(int(r))
        acc[i] += pc[r]
        cnt[i] += 1
        if cnt[i] == W2:
            closed.append(bins[i])
            bins[i] = []
            acc[i] = 0
            cnt[i] = 0
    closed.extend(b for b in bins if b)
    return closed


def _preprocess(x, edge_row, edge_col, edge_val):
    N, Cin = x.shape
    E = edge_row.shape[0]
    assert Cin == C and N % NCORES == 0
    R = N // NCORES

    vals = (-0.5 * edge_val).astype(np.float64)
    if E >= N and np.array_equal(edge_row[E - N:], np.arange(N, dtype=edge_row.dtype)) \
            and np.array_equal(edge_col[E - N:], edge_row[E - N:]):
        vals[E - N:] += 0.5
        er, ec = edge_row, edge_col
    else:
        er = np.concatenate([edge_row, np.arange(N, dtype=edge_row.dtype)])
        ec = np.concatenate([edge_col, np.arange(N, dtype=edge_col.dtype)])
        vals = np.concatenate([vals, np.full(N, 0.5, np.float64)])
        E = er.shape[0]
    vals = vals.astype(np.float32)

    order = np.argsort(er, kind="stable")
    er_s = er[order].astype(np.int64)
    ec_s = ec[order].astype(np.int64)
    vals_s = vals[order]
    deg = np.bincount(er, minlength=N).astype(np.int64)
    rowptr = np.zeros(N + 1, dtype=np.int64)
    np.cumsum(deg, out=rowptr[1:])

    src_owner = ec_s // R               # owner core of each edge's source
    src_quarter = src_owner >> 1        # quarter (core pair) of the source

    # per-row per-quarter counts, windows per core
    percore = []
    for c in range(NCORES):
        lo, hi = rowptr[c * R], rowptr[(c + 1) * R]
        rloc = (er_s[lo:hi] - c * R)
        q = src_quarter[lo:hi]
        pc = np.zeros((R, NQ), np.int64)
        np.add.at(pc, (rloc, q), 1)
        assert pc.max() <= SLOTS
        wins = _pack_windows(pc)
        assert len(wins) <= NV // W2, f"core {c}: {len(wins)} windows > cap"
        percore.append(wins)

    NW = max(len(w) for w in percore)
    NBR = (NW + BW - 1) // BW           # real psum batches per step
    NWP = NBR * BW                      # windows incl batch padding
    NV2 = ((NWP * W2 + CBR - 1) // CBR) * CBR
    NBC = NV2 // CBR                    # combination batches
    CS_ROWS = NBC * P88 * GC
    TPB = PSF // 128                    # 4

    # vrow maps
    vrow_of = np.full(N, -1, dtype=np.int64)
    vreal_all = []
    for c in range(NCORES):
        vreal = np.full(NV, -1, dtype=np.int64)
        for w, rows in enumerate(percore[c]):
            for lr, r in enumerate(rows):
                vrow_of[c * R + r] = w * W2 + lr
                vreal[w * W2 + lr] = c * R + r
        vreal_all.append(vreal)
    gidx_of = (np.arange(N) // R) * NV + vrow_of        # global table row
    assert (vrow_of[deg > 0] >= 0).all() and vrow_of.min() >= -1
    # every real row must be placed (self-loops -> deg>0 for all)
    assert vrow_of.min() >= 0

    data = []
    for c in range(NCORES):
        lo, hi = rowptr[c * R], rowptr[(c + 1) * R]
        e_r = er_s[lo:hi] - c * R
        e_gidx = gidx_of[ec_s[lo:hi]]
        e_q = (e_gidx >> 15).astype(np.int64)           # quarter 0..3
        e_lidx = (e_gidx & 32767).astype(np.int64)      # idx within quarter
        e_val = vals_s[lo:hi]
        e_w = vrow_of[er_s[lo:hi]] // W2                # window
        e_lr = vrow_of[er_s[lo:hi]] % W2                # row within window

        # group edges by (window, quarter); position within group
        key = e_w * NQ + e_q
        korder = np.argsort(key, kind="stable")
        ksorted = key[korder]
        gcnt = np.bincount(ksorted, minlength=NWP * NQ)
        assert gcnt.max() <= SLOTS
        goff = np.zeros(NWP * NQ + 1, np.int64)
        np.cumsum(gcnt, out=goff[1:])
        pos = np.arange(len(korder)) - goff[ksorted]    # slot within chunk

        w_s = e_w[korder]
        q_s = e_q[korder]
        b_s = w_s // BW
        wp_s = w_s % BW

        # idx stream: gather (b,q) covers BW*128 idxs; i = wp*128 + pos
        # wrap16: partition i%16 (x8 replicas), col (b*NQ+q)*128 + i//16
        ii = wp_s * SLOTS + pos
        idx_arr = np.zeros((16, NBR * NQ * 128), np.int16)
        idx_arr[ii % 16, (b_s * NQ + q_s) * 128 + ii // 16] = e_lidx[korder]
        idx_full = np.tile(idx_arr, (8, 1))

        # S: column j = b*64 + q*16 + wp ; S[pos, j*32 + lr] = val
        jcol = b_s * (NQ * BW) + q_s * BW + wp_s
        S = np.zeros((SLOTS, NBR * NQ * BW * W2), np.float32)
        S[pos, jcol * W2 + e_lr[korder]] = e_val[korder]
        S16 = S.astype(np.float16)

        # x in this core's vrow order (m=0 data)
        xv_own = np.zeros((NV2, C), dtype=np.float32)
        vr = vreal_all[c]
        nreal = min(NV, NV2)
        vrs = vr[:nreal]
        mask = vrs >= 0
        xv_own[:nreal][mask] = x[vrs[mask]]

        data.append(dict(idx=idx_full, S16=S16, xv_own=xv_own,
                         vreal=vr))

    # full padded gather table for step 1
    xv_full = np.zeros((NVG, CP), dtype=np.float32)
    for c in range(NCORES):
        vr = data[c]["vreal"]
        mask = vr >= 0
        xv_full[c * NV:(c + 1) * NV, :C][mask] = x[vr[mask]]

    coeffs = _bern_coeff_matrix(K)
    KR = np.zeros((P88, P88), dtype=np.float32)
    for rs_ in range(RS8):
        for kk in range(KD):
            for mm in range(KD):
                KR[rs_ * KD + mm, rs_ * KD + kk] = coeffs[kk, mm]

    geom = dict(N=N, R=R, NW=NW, NBR=NBR, NWP=NWP, NV2=NV2, NBC=NBC,
                CS_ROWS=CS_ROWS, TPB=TPB)
    return geom, data, KR, xv_full


def _build_program(geom):
    NBR, NBC = geom["NBR"], geom["NBC"]
    NV2, CS_ROWS, TPB = geom["NV2"], geom["CS_ROWS"], geom["TPB"]

    nc = bacc.Bacc("TRN2", target_bir_lowering=False, debug=False,
                   num_devices=NCORES, num_swdge_queues=4)
    f32, i16, fp16 = mybir.dt.float32, mybir.dt.int16, mybir.dt.float16

    xv_in = nc.dram_tensor("xv", [NVG, CP], f32, kind="ExternalInput").ap()
    xvo_in = nc.dram_tensor("xvo", [NV2, C], f32, kind="ExternalInput").ap()
    idx_in = nc.dram_tensor("idx", [128, NBR * NQ * 128], i16,
                            kind="ExternalInput").ap()
    S_in = nc.dram_tensor("S16", [SLOTS, NBR * NQ * BW * W2], fp16,
                          kind="ExternalInput").ap()
    KR_in = nc.dram_tensor("KR", [P88, P88], f32, kind="ExternalInput").ap()
    out_t = nc.dram_tensor("out", [RS8 * KD * NBC * GC, C], f32,
                           kind="ExternalOutput").ap()

    with tile.TileContext(nc) as tc:
        with tc.tile_pool(name="dramv", bufs=2, space="DRAM") as dramv, \
             tc.tile_pool(name="dramp", bufs=2, space="DRAM") as dramp, \
             tc.tile_pool(name="dramc", bufs=1, space="DRAM") as dramc, \
             tc.tile_pool(name="const", bufs=1) as constp, \
             tc.tile_pool(name="psum", bufs=2, space="PSUM") as psum, \
             tc.tile_pool(name="psumt", bufs=4, space="PSUM") as psumt:

            ident = constp.tile([48, 48], f32)
            make_identity(nc, ident)
            KR_t = constp.tile([P88, P88], f32)
            nc.sync.dma_start(KR_t[:], KR_in[:])

            C_spread = dramc.tile([CS_ROWS, C], f32)

            with tc.tile_pool(name="sbufA", bufs=3) as sbuf, \
                 tc.tile_pool(name="sbufG", bufs=3) as sbufg, \
                 tc.tile_pool(name="sbufI", bufs=2) as sbufi, \
                 tc.tile_pool(name="sbufR", bufs=1) as sbufr:

                S_sb = sbufr.tile([SLOTS, NBR * NQ * BW * W2], fp16)
                nc.sync.dma_start(S_sb[:], S_in[:])

                # ---- m=0: xv_own -> C_spread (affine) ----
                for b in range(NBC):
                    src = xvo_in[b * CBR:(b + 1) * CBR, :] \
                        .rearrange("(g rs) c -> rs g c", rs=RS8)
                    dst = C_spread[b * P88 * GC:(b + 1) * P88 * GC, :] \
                        .rearrange("(p g) c -> p g c", p=P88)[::KD, :, :]
                    nc.sync.dma_start(dst, src)

                # ---- K Laplacian power steps ----
                vprev = None
                for m in range(1, K + 1):
                    p_slab = dramp.tile([NV, CP], f32)
                    vsrc = xv_in if m == 1 else vprev[:, :]
                    for b in range(NBR):
                        ps = psum.tile([C, PSF], f32, tag="spmm")
                        idx_sb = sbufi.tile([128, NQ * 128], i16, tag="idx")
                        nc.sync.dma_start(
                            idx_sb[:],
                            idx_in[:, b * NQ * 128:(b + 1) * NQ * 128])
                        # <=1024 idxs per gather (65 descs/ring; 2048 would
                        # overflow the SWDGE descriptor ring and hang).
                        # One tile per (quarter, half) so the tile tracker
                        # doesn't serialize gathers on different queues.
                        HB = BW // 2
                        Vq = [None] * (NQ * 2)
                        for h in range(2):          # round-robin queues so the
                            for q in range(NQ):     # 4 Q7 pairs overlap
                                vt = sbufg.tile([SLOTS, HB * CP], f32,
                                                tag=f"vg{q}{h}")
                                nc.gpsimd.dma_gather(
                                    vt[:].rearrange("p (j e) -> p j e", e=CP),
                                    vsrc[q * 2 * NV:(q + 1) * 2 * NV, :],
                                    idx_sb[:, q * 128 + h * 64:q * 128 + (h + 1) * 64],
                                    HB * SLOTS, HB * SLOTS, CP, queue_num=q)
                                Vq[q * 2 + h] = vt
                        # fp16 S slice -> f32 (PE requires f32 x f32)
                        Sf = sbufi.tile([SLOTS, NQ * BW * W2], f32, tag="sf")
                        nc.scalar.copy(
                            Sf[:],
                            S_sb[:, b * NQ * BW * W2:(b + 1) * NQ * BW * W2])
                        for w in range(BW):
                            for q in range(NQ):
                                j = q * BW + w
                                vt = Vq[q * 2 + w // HB]
                                vv = vt[:].rearrange("p (j e) -> p j e", e=CP)
                                nc.tensor.matmul(
                                    out=ps[:, w * W2:(w + 1) * W2],
                                    lhsT=vv[:, w % HB, :C],
                                    rhs=Sf[:, j * W2:(j + 1) * W2],
                                    start=(q == 0), stop=(q == NQ - 1))
                        ps_sb = sbuf.tile([C, PSF], f32, tag="psdrain")
                        nc.scalar.copy(ps_sb[:], ps[:])
                        rowmaj = sbuf.tile([128, TPB * C], f32, tag="rowmaj")
                        for t in range(TPB):
                            pt = psumt.tile([128, C], f32, tag="ptrans")
                            nc.tensor.transpose(
                                out=pt[:], in_=ps_sb[:, t * 128:(t + 1) * 128],
                                identity=ident[:])
                            nc.scalar.copy(rowmaj[:, t * C:(t + 1) * C], pt[:])
                        # slab write (affine): vrow = b*PSF + t*128 + p
                        nc.sync.dma_start(
                            p_slab[b * PSF:(b + 1) * PSF, :C]
                            .rearrange("(t p) c -> p t c", p=128),
                            rowmaj[:].rearrange("p (t c) -> p t c", t=TPB))
                        # C_spread write (affine):
                        # flat = (b2*P88 + rs*KD + m)*GC + g2 ; vrow=b*PSF+t*128+p
                        # b2 = b//2 ; rs = p%8 ; g2 = (b%2)*64 + t*16 + p//8
                        base = ((b // 2) * P88 + m) * GC + (b % 2) * 64
                        cs_ap = C_spread[:, :]
                        for t in range(TPB):
                            dst = bass.AP(
                                cs_ap.tensor,
                                cs_ap.offset + (base + t * 16) * C,
                                [[C, 16], [KD * GC * C, 8], [1, C]])
                            nc.sync.dma_start(dst, rowmaj[:, t * C:(t + 1) * C])
                    if m < K:
                        vnew = dramv.tile([NVG, CP], f32, addr_space="Shared")
                        nc.gpsimd.collective_compute(
                            "AllGather", mybir.AluOpType.bypass,
                            replica_groups=[list(range(NCORES))],
                            ins=[p_slab[:, :]],
                            outs=[vnew[:, :]])
                        vprev = vnew

            # ---- Bernstein combination ----
            with tc.tile_pool(name="sbufB", bufs=2) as sbufb:
                for b in range(NBC):
                    rhs = sbufb.tile([P88, GC * C], f32, tag="crhs")
                    nc.sync.dma_start(
                        rhs[:],
                        C_spread[b * P88 * GC:(b + 1) * P88 * GC, :]
                        .rearrange("(p g) c -> p (g c)", p=P88))
                    outb = sbufb.tile([P88, GC * C], f32, tag="cout")
                    nmm = (GC * C + 511) // 512
                    for j in range(nmm):
                        f0 = j * 512
                        f1 = min(f0 + 512, GC * C)
                        cp = psum.tile([P88, 512], f32, tag="cpsum")
                        nc.tensor.matmul(out=cp[:, :f1 - f0], lhsT=KR_t[:],
                                         rhs=rhs[:, f0:f1], start=True, stop=True)
                        nc.scalar.copy(outb[:, f0:f1], cp[:, :f1 - f0])
                    dst = bass.AP(
                        out_t.tensor,
                        out_t.offset + b * GC * C,
                        [[KD * NBC * GC * C, RS8], [NBC * GC * C, KD], [1, GC * C]])
                    nc.sync.dma_start(dst, outb[:])

    nc.compile()
    return nc


def _make_in_maps(data, KR, xv_full):
    in_maps = []
    for d in data:
        in_maps.append({
            "xv": xv_full,
            "xvo": d["xv_own"],
            "idx": d["idx"],
            "S16": d["S16"],
            "KR": KR,
        })
    return in_maps


def _ensure_ntff_hook():
    try:
        from antenv.axon_hooks import get_axon_ntff_profile_hook  # noqa: F401
        return True
    except ImportError:
        pass
    try:
        import types
        import antenv
        from trn_agent_boot.trn_boot import _ntff_profile_via_ctypes
        mod = types.ModuleType("antenv.axon_hooks")
        _hook = [None]
        mod.set_axon_ntff_profile_hook = lambda h: _hook.__setitem__(0, h)
        mod.get_axon_ntff_profile_hook = lambda: _hook[0]
        sys.modules["antenv.axon_hooks"] = mod
        antenv.axon_hooks = mod
        mod.set_axon_ntff_profile_hook(
            _ntff_profile_via_ctypes("/opt/axon/libaxon_pjrt.so"))
        return True
    except Exception:
        return False


def kernel(x, edge_row, edge_col, edge_val):
    global LAST_EXEC_TIME_NS, LAST_RESULTS
    x = np.ascontiguousarray(np.asarray(x, dtype=np.float32))
    edge_row = np.asarray(edge_row, dtype=np.int32)
    edge_col = np.asarray(edge_col, dtype=np.int32)
    edge_val = np.asarray(edge_val, dtype=np.float32)
    N, Cin = x.shape

    geom, data, KR, xv_full = _preprocess(x, edge_row, edge_col, edge_val)
    nc = _build_program(geom)
    in_maps = _make_in_maps(data, KR, xv_full)

    if os.environ.get("K3_SIM"):
        from types import SimpleNamespace
        from concourse.bass_interp import MultiCoreSim
        sim = MultiCoreSim(nc, num_cores=NCORES,
                           require_finite=False, require_nnan=False)
        cores = list(sim.cores.values())
        for cid, core in enumerate(cores):
            for k2, v2 in in_maps[cid].items():
                core.tensor(k2)[:] = v2
        sim.simulate()
        res = SimpleNamespace(
            results=[{"out": np.asarray(core.tensor("out"))} for core in cores],
            exec_time_ns=None)
    else:
        trace = bool(os.environ.get("BASS_TRACE"))
        if trace:
            trace = _ensure_ntff_hook()
        res = bass_utils.run_bass_kernel_spmd(
            nc, in_maps, core_ids=list(range(NCORES)), trace=trace)
    LAST_RESULTS = res
    LAST_EXEC_TIME_NS = res.exec_time_ns

    NBC = geom["NBC"]
    out = np.empty((KD, N, C), dtype=np.float32)
    for c in range(NCORES):
        raw = res.results[c]["out"].reshape(RS8, KD, NBC, GC, C)
        vr = data[c]["vreal"]
        mask = vr >= 0
        vrows = np.nonzero(mask)[0]
        bb = vrows // CBR
        gg = (vrows % CBR) // RS8
        rs = vrows % RS8
        out[:, vr[mask], :] = raw[rs, :, bb, gg, :].transpose(1, 0, 2)
    return out

